# revision 31
# baseline (speedup 1.0000x reference)
"""CombinedSparsity (spatial max-pool + lifetime top-k + max-unpool) on 8 TRN2 cores.

Strategy: shard the 128 channels across 8 cores (16 each). Per (b, c) map the
output is all zeros except (possibly) one element: the map's max, written back
at its argmax position, kept only if that max is among the top-6 over the batch
for its channel. The kernel is HBM-read-bound (33.5MB/core); the stream must
stay saturated, so nothing mid-stream may fragment the load descriptors (ring
service is ~1.2us per descriptor: one per 32KB partition line), add ring
traffic, or make the in-order DVE stream wait on a small DMA:

  1. stream the shard in 2-channel groups; each group gets a TWO-LEVEL max
     reduce on DVE: HW=4096 -> 64 chunk-partials -> 1 pooled value, then a
     64-wide InstMaxIndex per channel (needle = its pooled max broadcast)
     finds each (b, c)'s argmax CHUNK in-stream — pure compute, no spill.
  2. per prep-slice of channels (6/6/2/1/1), find the per-channel top-6 batch
     entries (transpose + InstMax/InstMaxIndex) and compact (x_row, value)
     per survivor via DMA. The last channel's DMA is sub-split 4x so only
     ~1us of reduce remains when the final byte lands.
  3. survivors of channels 0-14 (90 of 96) are resolved DURING the final
     sub-streams: spill the chunk-index map, indirect-gather each survivor's
     chunk index (4B) then its 64-elem chunk from x, find the in-chunk
     position, scatter. Only ch15's 6 survivors run post-stream. Index math
     around gathers stays on GpSimd (no cross-engine semaphore hops); tail
     DVE ops are chain-ordered after the last sub-reduces so the list
     scheduler cannot stall the reduce stream on a gather.
     Output stays zero elsewhere (PJRT output buffers are donated zero-filled).
"""
import numpy as np

import concourse.bass as bass
import concourse.bacc as bacc
import concourse.tile as tile
from concourse import mybir
from concourse.bass_utils import run_bass_kernel_spmd
from concourse.masks import make_identity

B = 128
C_FULL = 128
H = 64
W = 64
HW = H * W
N_CORES = 8
CSH = C_FULL // N_CORES      # channels per core
K = 6                        # lifetime top-k
S = 64                       # chunks per map (two-level reduce)
T = HW // S                  # elems per chunk
F32 = mybir.dt.float32
I32 = mybir.dt.int32
U32 = mybir.dt.uint32

PREPS = [(0, 6), (6, 12), (12, 14), (14, 15), (15, 16)]
NPRE = 15 * K                # survivors resolved mid-stream (channels 0-14)
NPOST = K                    # survivors of the final channel
N_SUB = 4                    # sub-DMAs for the final channel

_nc_cache = None


def _build():
    global _nc_cache
    if _nc_cache is not None:
        return _nc_cache

    nc = bacc.Bacc("TRN2", target_bir_lowering=False, debug=False)
    x = nc.dram_tensor("x", [B, CSH, HW], F32, kind="ExternalInput")
    y = nc.dram_tensor("y", [B, CSH, HW], F32, kind="ExternalOutput")
    x64 = x.rearrange("b c (s t) -> (b c s) t", t=T)   # chunk-row view
    y_elem = y.rearrange("b c h -> (b c h)")[:, None]
    n_elem = B * CSH * HW

    with tile.TileContext(nc) as tc:
        def chain(binst):
            """No-op: the merged tail is dependency-ordered after the last
            prep's compact, so no manual DVE ordering is needed."""
            return binst

        with (
            tc.tile_pool(name="const", bufs=1) as cp,
            tc.tile_pool(name="gxp", bufs=5) as gxp,
            tc.tile_pool(name="small", bufs=1) as sp,
            tc.tile_pool(name="dram", bufs=1, space="DRAM") as dp,
            tc.tile_pool(name="ps", bufs=1, space="PSUM") as pp,
        ):
            # NOTE: the const pool must hold EXACTLY these two 512B tiles:
            # anything extra shifts the gxp pool's SBUF base and fragments
            # the stream loads' 32KB line descriptors (~25% ring-time cost).
            ident0 = cp.tile([B, B], F32)
            make_identity(nc, ident0[:])
            # keep matmul inputs single-producer-engine (DVE)
            ident = cp.tile([B, B], F32)
            nc.vector.tensor_copy(out=ident[:], in_=ident0[:])

            # per-prep absolute channel-index columns (scalar operands must
            # live in the same partitions as the op's lanes, i.e. start at 0)
            abs_cols = []
            for p, (c_lo, c_hi) in enumerate(PREPS):
                n = c_hi - c_lo
                abs_i = sp.tile([n, 1], I32, name=f"absi{p}")
                nc.gpsimd.iota(
                    abs_i[:], pattern=[[1, 1]], base=c_lo, channel_multiplier=1
                )
                abs_f = sp.tile([n, 1], F32, name=f"absf{p}")
                nc.vector.tensor_copy(out=abs_f[:], in_=abs_i[:])
                abs_cols.append(abs_f)

            cpk_all = sp.tile([CSH * K, 2], F32)  # (x_row, value) per survivor
            jc_all = sp.tile([B, CSH], F32)       # argmax chunk per (b, c)

            def emit_prep(p, c_lo, c_hi):
                n = c_hi - c_lo
                last = p == len(PREPS) - 1
                # pin DVE order for the last prep's compute so the tail finds
                # can't be scheduled ahead of it
                ch = chain if last else (lambda _: None)
                partials = sp.tile([B, n * S], F32, name=f"part{p}")
                pooled = sp.tile([B, n], F32, name=f"pooled{p}")
                jc8 = sp.tile([B, n * 8], U32, name=f"jc8{p}")

                def finish_group(crel, ncols):
                    ch(nc.vector.tensor_reduce(
                        out=pooled[:, crel:crel + ncols],
                        in_=partials[:, crel * S:(crel + ncols) * S].rearrange(
                            "p (c s) -> p c s", c=ncols
                        ),
                        axis=mybir.AxisListType.X,
                        op=mybir.AluOpType.max,
                    ))
                    # in-stream argmax chunk per channel: pure DVE, no DMA dep
                    for c in range(crel, crel + ncols):
                        ch(nc.vector.max_index(
                            out=jc8[:, c * 8:(c + 1) * 8],
                            in_max=pooled[:, c:c + 1].to_broadcast([B, 8]),
                            in_values=partials[:, c * S:(c + 1) * S],
                        ))

                # ---- streaming ----
                if not last:
                    for crel in range(0, n, 2):
                        ncols = min(2, n - crel)
                        c0 = c_lo + crel
                        gx = gxp.tile([B, ncols * HW], F32, tag="gx")
                        nc.sync.dma_start(out=gx[:], in_=x[:, c0:c0 + ncols, :])
                        nc.vector.tensor_reduce(
                            out=partials[:, crel * S:(crel + ncols) * S],
                            in_=gx[:].rearrange(
                                "p (c s t) -> p c s t", c=ncols, s=S
                            ),
                            axis=mybir.AxisListType.X,
                            op=mybir.AluOpType.max,
                        )
                        finish_group(crel, ncols)
                else:
                    # final channel: sub-split so its reduce isn't exposed
                    sub = HW // N_SUB
                    for j in range(N_SUB):
                        gx = gxp.tile([B, sub], F32, tag="gx")
                        nc.sync.dma_start(
                            out=gx[:], in_=x[:, c_lo:c_lo + 1,
                                             j * sub:(j + 1) * sub]
                        )
                        ch(nc.vector.tensor_reduce(
                            out=partials[:, j * (sub // T):(j + 1) * (sub // T)],
                            in_=gx[:].rearrange("p (s t) -> p s t", t=T),
                            axis=mybir.AxisListType.X,
                            op=mybir.AluOpType.max,
                        ))
                    finish_group(0, 1)

                # pack chunk indices into the shared [B, CSH] map
                ch(nc.vector.tensor_copy(
                    out=jc_all[:, c_lo:c_hi],
                    in_=jc8[:].rearrange("q (c j) -> q c j", j=8)[:, :, 0:1],
                ))

                # ---- prep: top-8 over batch, compact survivors ----
                pooled_t_ps = pp.tile([n, B], F32, name=f"ptps{p}")
                nc.tensor.transpose(
                    out=pooled_t_ps[:], in_=pooled[:], identity=ident[:]
                )
                pooled_t = sp.tile([n, B], F32, name=f"pt{p}")
                nc.scalar.copy(out=pooled_t[:], in_=pooled_t_ps[:])

                pt8 = sp.tile([n, 8], F32, name=f"pt8{p}")
                ch(nc.vector.max(out=pt8[:], in_=pooled_t[:]))
                pi8 = sp.tile([n, 8], U32, name=f"pi8{p}")
                ch(nc.vector.max_index(
                    out=pi8[:], in_max=pt8[:], in_values=pooled_t[:]
                ))
                pi8f = sp.tile([n, 8], F32, name=f"pi8f{p}")
                ch(nc.vector.tensor_copy(out=pi8f[:], in_=pi8[:]))

                pk = sp.tile([n, 8 * 2], F32, name=f"pk{p}")
                pkv = pk[:].rearrange("q (j k) -> q j k", k=2)
                ch(nc.vector.tensor_scalar(
                    out=pkv[:, :, 0:1], in0=pi8f[:], scalar1=float(CSH),
                    scalar2=abs_cols[p][:, 0:1],
                    op0=mybir.AluOpType.mult, op1=mybir.AluOpType.add,
                ))
                nc.scalar.copy(out=pkv[:, :, 1:2], in_=pt8[:])

                nc.gpsimd.dma_start(
                    out=cpk_all[c_lo * K:c_hi * K, :], in_=pkv[:, 0:K, :]
                )

            def resolve(cpk, nsv, jc_src, vb, find_chain):
                """jc gather -> chunk gather -> in-chunk find -> scatter.
                Index math stays on GpSimd (no cross-engine hops around the
                gathers); the DVE ops go through find_chain for ordering."""
                rows_i = sp.tile([nsv, 1], I32)
                nc.vector.tensor_copy(out=rows_i[:], in_=cpk[:, 0:1])
                jcs = sp.tile([nsv, 1], F32)
                nc.gpsimd.indirect_dma_start(
                    out=jcs[:], out_offset=None,
                    in_=jc_src[:].rearrange("b c -> (b c)")[:, None],
                    in_offset=bass.IndirectOffsetOnAxis(
                        ap=rows_i[:, 0:1], axis=0
                    ),
                )
                rows2 = sp.tile([nsv, 1], F32)
                nc.vector.tensor_scalar(
                    out=rows2[:], in0=cpk[:, 0:1], scalar1=float(S),
                    scalar2=jcs[:, 0:1],
                    op0=mybir.AluOpType.mult, op1=mybir.AluOpType.add,
                )
                rows2_i = sp.tile([nsv, 1], I32)
                nc.vector.tensor_copy(out=rows2_i[:], in_=rows2[:])
                ck = sp.tile([nsv, T], F32)
                nc.gpsimd.indirect_dma_start(
                    out=ck[:], out_offset=None,
                    in_=x64[:],
                    in_offset=bass.IndirectOffsetOnAxis(
                        ap=rows2_i[:, 0:1], axis=0
                    ),
                )
                t8 = sp.tile([nsv, 8], U32)
                find_chain(nc.vector.max_index(
                    out=t8[:], in_max=vb[:], in_values=ck[:]
                ))
                tf = sp.tile([nsv, 1], F32)
                find_chain(nc.vector.tensor_copy(out=tf[:], in_=t8[:, 0:1]))
                off_f = sp.tile([nsv, 1], F32)
                find_chain(nc.vector.tensor_scalar(
                    out=off_f[:], in0=rows2[:], scalar1=float(T),
                    scalar2=tf[:, 0:1],
                    op0=mybir.AluOpType.mult, op1=mybir.AluOpType.add,
                ))
                off_i = sp.tile([nsv, 1], I32)
                find_chain(nc.vector.tensor_copy(out=off_i[:], in_=off_f[:]))
                nc.gpsimd.indirect_dma_start(
                    out=y_elem[:],
                    out_offset=bass.IndirectOffsetOnAxis(
                        ap=off_i[:, 0:1], axis=0
                    ),
                    in_=cpk[:, 1:2],
                    in_offset=None,
                    bounds_check=n_elem - 1,
                    oob_is_err=False,
                )

            for p, (c_lo, c_hi) in enumerate(PREPS):
                emit_prep(p, c_lo, c_hi)

            # merged post-stream tail over all 96 survivors
            jc_d = dp.tile([B, CSH], F32)
            nc.scalar.dma_start(out=jc_d[:], in_=jc_all[:])
            vb = sp.tile([CSH * K, 8], F32)
            chain(nc.vector.tensor_copy(
                out=vb[:], in_=cpk_all[:, 1:2].to_broadcast([CSH * K, 8])
            ))
            resolve(cpk_all, CSH * K, jc_d, vb, chain)

    nc.finalize()
    _nc_cache = nc
    return nc


def _install_profile_hook():
    """Inject the antenv.axon_hooks shim so trace=True captures NTFFs."""
    import sys
    import types

    if "antenv.axon_hooks" in sys.modules:
        return
    import antenv
    import trn_agent_boot.trn_boot as tb

    mod = types.ModuleType("antenv.axon_hooks")
    mod._hook = tb._ntff_profile_via_ctypes("/opt/axon/libaxon_pjrt.so")
    mod.get_axon_ntff_profile_hook = lambda: mod._hook
    mod.set_axon_ntff_profile_hook = lambda h: setattr(mod, "_hook", h)
    sys.modules["antenv.axon_hooks"] = mod
    antenv.axon_hooks = mod

    # no S3 in this container — keep artifacts local
    import concourse.bass_utils as bu

    bu.upload_artifacts = lambda tmpdir: tmpdir


def run(activations, trace=False):
    if trace:
        _install_profile_hook()
    act = np.asarray(activations)
    assert act.shape == (B, C_FULL, H, W), act.shape
    act = act.astype(np.float32, copy=False)
    nc = _build()
    in_maps = [
        {"x": np.ascontiguousarray(act[:, i * CSH:(i + 1) * CSH]).reshape(B, CSH, HW)}
        for i in range(N_CORES)
    ]
    res = run_bass_kernel_spmd(
        nc, in_maps, core_ids=list(range(N_CORES)), trace=trace
    )
    out = np.concatenate(
        [r["y"].reshape(B, CSH, H, W) for r in res.results], axis=1
    )
    return out, res


def kernel(activations):
    out, _ = run(activations, trace=False)
    return out
